# revision 3
# baseline (speedup 1.0000x reference)
"""Trainium2 Bass kernel for nn_Blender (per-style MLP blender).

Strategy
--------
Pure data parallel over the batch: each of the 8 NeuronCores processes
B/8 = 1024 samples with a full replica of the weights. No collectives.

On-chip layout is feature-major ([features -> partitions, batch -> free
dim]) so every GEMM contracts along the partition axis with batch as the
moving dim (N=512 = one fp32 PSUM bank). The host pre-transposes
global_styles to [S, D, B] (fp16) and post-transposes the output back,
so all device DMA is contiguous.

GEMMs run in fp16 (1 cycle/row, background-buffer weight loads)
accumulating into fp32 PSUM; epilogues (bias/relu/residual) run on
ACT/DVE.

Algebraic folds (all exact):
  * bn2 + gm1 fuse: gm1_in = concat_s(h1_s @ bn_w2_s + bn_b2_s), so
    gm1_out = sum_s h1_s @ (bn_w2_s @ gm_w1_block_s) + folded bias.
    One block-stacked K=128 GEMM per 4-style group replaces bn2+gm1.
  * gm2 fold: fc1 consumes gmh (the gm hidden) directly through
    W_g' = gm_w2 @ fc_w1[:, :GH]; gm_b2's term goes into fc1's bias.
  * age rank-2: with age_b1 == 0 and ages >= 0 the age MLP is exactly
    affine in the scalar age: af = age*c + c0 (validated at runtime via
    lstsq; falls back to an explicit K=16 k-tile otherwise). Its fc1
    contribution is a rank-1 outer product age (x) d_s added into PSUM
    by the (otherwise idle) Vector engine, and a bias fold. This keeps
    every fc1 k-tile a full 128-row weight load, which the PE can
    preload into the background weight buffer while the previous
    matmul streams -- partial-row LDWEIGHTS cannot be pulled ahead and
    stall the PE ~400ns each.

Pipeline per core (BC=1024 samples, chunks of NB=512):
  phase 1: per style group (4 styles column-tiled into the 128-wide PE
           array, kt-major so the 4 col-groups stream concurrently):
           bn1 512->32 + fused bn2gm1 accumulation -> gmh [128, NB].
  phase 2: per style: fc1 = 4 gs k-tiles + 1 gmh k-tile -> +age rank-1
           (DVE) -> ReLU+bias (ACT) -> fc2 (4x4 k-tiles) -> bias +
           residual(gs) (DVE) -> yT fp16.
           gs tiles for the first STASH_S styles stay resident in SBUF
           from phase 1 (no second HBM read).
"""

import numpy as np

import concourse.bacc as bacc
import concourse.tile as tile
from concourse import mybir
from concourse.bass_utils import run_bass_kernel_spmd

S, D, BN, GH, AH, FCH = 18, 512, 32, 128, 16, 512
B = 8192
N_CORES = 8
BC = B // N_CORES          # samples per core
NB = 512                   # moving-dim (batch) tile = one fp32 PSUM bank
N_CHUNKS = BC // NB
GROUPS = [(0, 4), (4, 4), (8, 4), (12, 4), (16, 2)]
NG = len(GROUPS)
KT1 = 5                    # fc1 k-tiles: 4x gs(128) + gmh(128)
STASH_S = 14               # styles whose gs tiles stay resident across phases

F32 = mybir.dt.float32
F16 = mybir.dt.float16
MM_DT = mybir.dt.float16
NP_MM = np.float16

_CACHE = {}


def build_program(rank2: bool = True):
    nc = bacc.Bacc("TRN2", target_bir_lowering=False, debug=False,
                   num_devices=N_CORES)
    mm = nc.tensor.matmul

    din = lambda name, shape, dt=MM_DT: nc.dram_tensor(name, shape, dt, kind="ExternalInput").ap()
    gsT = din("gsT", [S, D, BC])
    bn_w1t = din("bn_w1t", [128, S * 4 * BN])
    bn_b1g = din("bn_b1g", [128, NG], F32)
    vg = din("vg", [128, NG * GH])
    gm_b1p = din("gm_b1p", [GH, 1], F32)
    fc_w1t = din("fc_w1t", [S, 128, KT1 * FCH])     # [s, p, kt*512 + h]
    fc_b1t = din("fc_b1t", [S, 128, 4], F32)
    fc_w2t = din("fc_w2t", [S, 128, 16 * 128])      # [s, p, (kt*4+dt)*128 + j]
    fc_b2t = din("fc_b2t", [S, 128, 4], F32)
    if rank2:
        ageB = din("ageB", [128, BC])               # age broadcast over partitions
        d_all = din("d_all", [128, S * 4], F32)     # rank-1 age dirs, [p, s*4+ht]
    else:
        afT = din("afT", [AH, BC])
        fa_w = din("fa_w", [AH, S * 4 * 128])       # af k-tile weights [a, (s,ht,j)]
    yT = nc.dram_tensor("yT", [S, D, BC], F16, kind="ExternalOutput").ap()

    Relu = mybir.ActivationFunctionType.Relu
    ADD = mybir.AluOpType.add
    MULT = mybir.AluOpType.mult

    with (
        tile.TileContext(nc) as tc,
        tc.tile_pool(name="consts", bufs=1) as consts,
        tc.tile_pool(name="stash", bufs=1) as stash_pool,
        tc.tile_pool(name="gstr", bufs=2) as gstr_pool,       # streamed gs (styles >= STASH_S)
        tc.tile_pool(name="act1", bufs=3) as act1_pool,
        tc.tile_pool(name="wp", bufs=2) as w_pool,
        tc.tile_pool(name="y1p", bufs=2) as y1_pool,
        tc.tile_pool(name="outp", bufs=4) as out_pool,
        tc.tile_pool(name="ps", bufs=1, space="PSUM") as ps,
    ):
        # ---- resident constants ----
        bn_w1_sb = consts.tile([128, S * 4 * BN], MM_DT, tag="bn_w1")
        nc.sync.dma_start(bn_w1_sb[:], bn_w1t[:])
        bn_b1_sb = consts.tile([128, NG], F32, tag="bn_b1")
        nc.sync.dma_start(bn_b1_sb[:], bn_b1g[:])
        vg_sb = consts.tile([128, NG * GH], MM_DT, tag="vg")
        nc.sync.dma_start(vg_sb[:], vg[:])
        gm_b1_sb = consts.tile([GH, 1], F32, tag="gm_b1")
        nc.sync.dma_start(gm_b1_sb[:], gm_b1p[:])
        if rank2:
            age_sb = consts.tile([128, BC], MM_DT, tag="ageB")
            nc.sync.dma_start(age_sb[:], ageB[:])
            d_sb = consts.tile([128, S * 4], F32, tag="d_all")
            nc.sync.dma_start(d_sb[:], d_all[:])
        else:
            af_sb = consts.tile([AH, BC], MM_DT, tag="af")
            nc.sync.dma_start(af_sb[:], afT[:])
            fa_sb = consts.tile([AH, S * 4 * 128], MM_DT, tag="fa_w")
            nc.sync.dma_start(fa_sb[:], fa_w[:])
        gmh_sb = [consts.tile([GH, NB], MM_DT, tag=f"gmh{c}", name=f"gmh{c}")
                  for c in range(N_CHUNKS)]

        def load_gs(s, c, pool, tag):
            b0 = c * NB
            t = pool.tile([128, 4 * NB], MM_DT, tag=tag, name=f"gs_{s}_{c}_{tag}")
            for kt in range(4):     # split per k-slice: first MMs start sooner
                nc.sync.dma_start(t[:, kt * NB:(kt + 1) * NB],
                                  gsT[s, kt * 128:(kt + 1) * 128, b0:b0 + NB])
            return t

        # ---------------- phase 1: bn1 + fused bn2gm1 -> gmh ----------------
        gs_tiles = {}      # (s, c) -> [128, 4*NB] tile
        for c in range(N_CHUNKS):
            ps_g1 = ps.tile([GH, NB], F32, tag="g1", bufs=2, name=f"ps_g1_{c}")
            for gi, (s0, ng) in enumerate(GROUPS):
                pN = 32 * ng
                gts = []
                for j in range(ng):
                    s = s0 + j
                    if s < STASH_S:
                        t = load_gs(s, c, stash_pool, f"gs_{s}_{c}")
                    else:
                        t = load_gs(s, c, gstr_pool, "gsS")
                    gs_tiles[(s, c)] = t
                    gts.append(t)
                ps_h1 = ps.tile([128, NB], F32, tag="h1", bufs=2,
                                name=f"ps_h1_{gi}_{c}")
                for kt in range(4):         # kt-major: col-groups concurrent
                    for j in range(ng):
                        s = s0 + j
                        mm(ps_h1[32 * j:32 * j + 32, :],
                           bn_w1_sb[:, (s * 4 + kt) * BN:(s * 4 + kt + 1) * BN],
                           gts[j][:, kt * NB:(kt + 1) * NB],
                           start=(kt == 0), stop=(kt == 3),
                           tile_position=(0, 32 * j))
                h1 = act1_pool.tile([128, NB], MM_DT, tag="h1s", name=f"h1_{gi}_{c}")
                nc.scalar.activation(h1[:pN, :], ps_h1[:pN, :], Relu,
                                     bias=bn_b1_sb[:pN, gi:gi + 1])
                mm(ps_g1[:], vg_sb[:pN, gi * GH:(gi + 1) * GH], h1[:pN, :],
                   start=(gi == 0), stop=(gi == NG - 1))
            nc.scalar.activation(gmh_sb[c][:], ps_g1[:], Relu, bias=gm_b1_sb[:])

        # ---------------- phase 2: per-style fc MLP + residual ----------------
        for s in range(S):
            w1s = w_pool.tile([128, KT1 * FCH], MM_DT, tag="w1", name=f"w1_{s}")
            nc.scalar.dma_start(w1s[:], fc_w1t[s, :, :])
            w2s = w_pool.tile([128, 16 * 128], MM_DT, tag="w2", name=f"w2_{s}")
            nc.scalar.dma_start(w2s[:], fc_w2t[s, :, :])
            b1s = w_pool.tile([128, 4], F32, tag="b1", name=f"b1_{s}")
            nc.scalar.dma_start(b1s[:], fc_b1t[s, :, :])
            b2s = w_pool.tile([128, 4], F32, tag="b2", name=f"b2_{s}")
            nc.scalar.dma_start(b2s[:], fc_b2t[s, :, :])

            for c in range(N_CHUNKS):
                b0 = c * NB
                if s < STASH_S:
                    gs_sb = gs_tiles[(s, c)]
                else:
                    gs_sb = load_gs(s, c, gstr_pool, "gsS")
                y1 = []
                for ht in range(4):
                    h0 = ht * 128
                    ps_y1 = ps.tile([128, NB], F32, tag="y1", bufs=2,
                                    name=f"ps_y1_{s}_{c}_{ht}")
                    for kt in range(4):      # gs k-tiles first (no gmh dep)
                        mm(ps_y1[:],
                           w1s[:, kt * FCH + h0:kt * FCH + h0 + 128],
                           gs_sb[:, kt * NB:(kt + 1) * NB],
                           start=(kt == 0), stop=False)
                    if not rank2:
                        mm(ps_y1[:],         # af k-tile (K=16)
                           fa_sb[:, (s * 4 + ht) * 128:(s * 4 + ht + 1) * 128],
                           af_sb[:, b0:b0 + NB],
                           start=False, stop=False)
                    mm(ps_y1[:],             # gmh k-tile last
                       w1s[:, 4 * FCH + h0:4 * FCH + h0 + 128],
                       gmh_sb[c][:],
                       start=False, stop=True)
                    if rank2:                # += age (x) d_s,ht  (rank-1, DVE)
                        nc.vector.scalar_tensor_tensor(
                            ps_y1[:], age_sb[:, b0:b0 + NB],
                            d_sb[:, s * 4 + ht:s * 4 + ht + 1],
                            ps_y1[:], op0=MULT, op1=ADD)
                    y1t = y1_pool.tile([128, NB], MM_DT, tag=f"y1_{ht}",
                                       name=f"y1_{s}_{c}_{ht}")
                    nc.scalar.activation(y1t[:], ps_y1[:], Relu, bias=b1s[:, ht:ht + 1])
                    y1.append(y1t)
                for dt_ in range(4):
                    ps_y = ps.tile([128, NB], F32, tag="y", bufs=2,
                                   name=f"ps_y_{s}_{c}_{dt_}")
                    for kt in range(4):
                        mm(ps_y[:],
                           w2s[:, (kt * 4 + dt_) * 128:(kt * 4 + dt_ + 1) * 128],
                           y1[kt][:],
                           start=(kt == 0), stop=(kt == 3))
                    o = out_pool.tile([128, NB], F16, tag="o", name=f"o_{s}_{c}_{dt_}")
                    nc.vector.scalar_tensor_tensor(
                        o[:], ps_y[:], b2s[:, dt_:dt_ + 1],
                        gs_sb[:, dt_ * NB:(dt_ + 1) * NB], op0=ADD, op1=ADD)
                    nc.gpsimd.dma_start(yT[s, dt_ * 128:(dt_ + 1) * 128, b0:b0 + NB], o[:])

    nc.compile()
    return nc


def _prep_weights(bn_w1, bn_b1, bn_w2, bn_b2, gm_w1, gm_b1, gm_w2, gm_b2,
                  fc_w1, fc_b1, fc_w2, fc_b2, c_age, c0_age, rank2):
    f = np.float32
    h = NP_MM
    # [p, (s, kt, j)] : bn_w1[s, kt*128+p, j]
    bn_w1t = np.ascontiguousarray(
        bn_w1.reshape(S, 4, 128, BN).transpose(2, 0, 1, 3).reshape(128, S * 4 * BN), h)
    bn_b1g = np.zeros((128, NG), f)
    for gi, (s0, ng) in enumerate(GROUPS):
        for j in range(ng):
            bn_b1g[32 * j:32 * j + 32, gi] = bn_b1[s0 + j]
    # fused bn2 @ gm_w1-block, group-stacked: vg[32j:+32, gi*128:+128] = V[s0+j]
    gm_w1b = gm_w1.reshape(S, BN, GH).astype(f)
    V = np.einsum('skm,smg->skg', bn_w2.astype(f), gm_w1b)      # [S, 32, 128]
    vg = np.zeros((128, NG * GH), h)
    for gi, (s0, ng) in enumerate(GROUPS):
        for j in range(ng):
            vg[32 * j:32 * j + 32, gi * GH:(gi + 1) * GH] = V[s0 + j]
    gm_b1p = (gm_b1.astype(f)
              + np.einsum('sm,smg->g', bn_b2.astype(f), gm_w1b)).reshape(GH, 1)

    W_g = fc_w1[:, :GH, :].astype(f)                            # [S, 128, 512]
    W_a = fc_w1[:, GH:GH + AH, :].astype(f)                     # [S, 16, 512]
    # fc1 rows: [gs (4x128) | gmh (gm_w2 @ W_g)]
    w1p = np.empty((S, KT1 * 128, FCH), f)
    w1p[:, :4 * 128] = fc_w1[:, GH + AH:]
    w1p[:, 4 * 128:] = np.einsum('kj,sjf->skf', gm_w2.astype(f), W_g)
    fc_w1t = np.ascontiguousarray(
        w1p.reshape(S, KT1, 128, FCH).transpose(0, 2, 1, 3).reshape(S, 128, KT1 * FCH), h)
    b1_full = (fc_b1.astype(f)
               + np.einsum('j,sjf->sf', gm_b2.astype(f), W_g))
    if rank2:
        b1_full = b1_full + np.einsum('a,saf->sf', c0_age, W_a)
    fc_b1t = np.ascontiguousarray(b1_full.reshape(S, 4, 128).transpose(0, 2, 1), f)
    fc_w2t = np.ascontiguousarray(
        fc_w2.reshape(S, 4, 128, 4, 128).transpose(0, 2, 1, 3, 4).reshape(S, 128, 16 * 128), h)
    fc_b2t = np.ascontiguousarray(fc_b2.reshape(S, 4, 128).transpose(0, 2, 1), f)
    out = dict(bn_w1t=bn_w1t, bn_b1g=bn_b1g, vg=vg, gm_b1p=gm_b1p,
               fc_w1t=fc_w1t, fc_b1t=fc_b1t, fc_w2t=fc_w2t, fc_b2t=fc_b2t)
    if rank2:
        d = np.einsum('a,saf->sf', c_age, W_a)                  # [S, 512]
        out["d_all"] = np.ascontiguousarray(
            d.reshape(S, 4, 128).transpose(2, 0, 1).reshape(128, S * 4), f)
    else:
        # af k-tile weights: fa_w[a, (s*4+ht)*128 + j] = W_a[s, a, ht*128+j]
        out["fa_w"] = np.ascontiguousarray(
            W_a.reshape(S, AH, 4, 128).transpose(1, 0, 2, 3).reshape(AH, S * 4 * 128), h)
    return out


def run(inputs: dict, trace: bool = False):
    """Build in_maps from full inputs, run SPMD on 8 cores, return
    (full_output, BassKernelResults)."""
    gs = inputs["global_styles"]
    ages = np.asarray(inputs["target_ages"], np.float32)
    # host: exact fp32 age MLP (tiny), then affine-in-age fit
    af = np.maximum(ages[:, None] @ inputs["age_w1"] + inputs["age_b1"], 0.0)
    af = (af @ inputs["age_w2"] + inputs["age_b2"]).astype(np.float32)  # [B, 16]
    A = np.stack([ages, np.ones_like(ages)], axis=1)                    # [B, 2]
    sol, *_ = np.linalg.lstsq(A.astype(np.float64), af.astype(np.float64),
                              rcond=None)
    resid = np.abs(af - (A @ sol.astype(np.float32))).max()
    rank2 = bool(resid <= 1e-4 * max(1.0, np.abs(af).max()))
    c_age, c0_age = sol[0].astype(np.float32), sol[1].astype(np.float32)

    key = ("nc", rank2)
    if key not in _CACHE:
        _CACHE[key] = build_program(rank2)
    nc = _CACHE[key]

    w = _prep_weights(
        inputs["bn_w1"], inputs["bn_b1"], inputs["bn_w2"], inputs["bn_b2"],
        inputs["gm_w1"], inputs["gm_b1"], inputs["gm_w2"], inputs["gm_b2"],
        inputs["fc_w1"], inputs["fc_b1"], inputs["fc_w2"], inputs["fc_b2"],
        c_age, c0_age, rank2)

    gsT_full = np.ascontiguousarray(gs.transpose(1, 2, 0).astype(NP_MM))  # [S, D, B]
    ages16 = ages.astype(NP_MM)
    afT_full = np.ascontiguousarray(af.T.astype(NP_MM))                   # [16, B]
    in_maps = []
    for c in range(N_CORES):
        sl = slice(c * BC, (c + 1) * BC)
        m = dict(w)
        m["gsT"] = np.ascontiguousarray(gsT_full[:, :, sl])
        if rank2:
            m["ageB"] = np.ascontiguousarray(
                np.broadcast_to(ages16[None, sl], (128, BC)))
        else:
            m["afT"] = np.ascontiguousarray(afT_full[:, sl])
        in_maps.append(m)

    res = run_bass_kernel_spmd(nc, in_maps, core_ids=list(range(N_CORES)),
                               trace=trace)
    yT = np.concatenate([res.results[c]["yT"][:, :, :] for c in range(N_CORES)],
                        axis=2)                              # [S, D, B] fp16
    y = yT.astype(np.float32).transpose(2, 0, 1)             # [B, S, D]
    return np.ascontiguousarray(y), res


def kernel(**inputs) -> np.ndarray:
    y, _ = run(inputs, trace=False)
    return y


# revision 4
# speedup vs baseline: 1.1455x; 1.1455x over previous
"""Trainium2 Bass kernel for nn_Blender (per-style MLP blender).

Strategy
--------
Pure data parallel over the batch: each of the 8 NeuronCores processes
B/8 = 1024 samples with a full replica of the weights. No collectives.

On-chip layout is feature-major ([features -> partitions, batch -> free
dim]) so every GEMM contracts along the partition axis with batch as the
moving dim (N=512 = one fp32 PSUM bank). The host pre-transposes
global_styles to [S, D, B] (fp16) and post-transposes the output back,
so all device DMA is contiguous. Output is written fp16 (adds ~5e-4
max-rel-err; tolerance is 2e-2).

GEMMs run in fp16 (1 cycle/row) accumulating into fp32 PSUM; epilogues
(bias/relu/residual) run on ACT/DVE. Every fc k-tile is a full 128-row
weight load so the PE preloads it into the background weight buffer
while the previous matmul streams (partial-row LDWEIGHTS cannot be
pulled ahead and stall the PE ~400ns each -- measured).

Algebraic folds (all exact):
  * bn2 + gm1 fuse: gm1_in = concat_s(h1_s @ bn_w2_s + bn_b2_s), so
    gm1_out = sum_s h1_s @ (bn_w2_s @ gm_w1_block_s) + folded bias.
    One block-stacked K=128 GEMM per 4-style group replaces bn2+gm1.
  * gm2 fold: fc1 consumes gmh (the gm hidden) directly through
    W_g' = gm_w2 @ fc_w1[:, :GH]; gm_b2's term goes into fc1's bias.
  * age rank-2: with age_b1 == 0 and ages >= 0 the age MLP is exactly
    affine in the scalar age: af = age*c + c0 (validated at runtime via
    lstsq; falls back to an explicit K=16 k-tile otherwise). Its fc1
    contribution is a rank-1 outer product age (x) d_s added into PSUM
    by the (otherwise idle) Vector engine, and a bias fold.

Schedule (chunk-outer, BC=1024 samples in 2 chunks of NB=512):
  phase1(c0)            -- DMA-paced front: 9.4 MB of gs for chunk 0
  fc pass c0, s=0..17   -- PE-bound; per-style weights double-buffered
    (phase1(c1) emitted after s=4: its gs DMAs stream during the c0
     pass; ~6us of PE work dropped into a PE-bound region)
  fc pass c1, s=0..17
Weight DMAs ride the gpsimd queue (idle early -> style-0 weights land
during phase 1; sync queue is saturated issuing gs loads at kernel
start). gs loads are one 3D-AP DMA per [128, 4*NB] tile (4 KB/partition
packets sustain ~385 GB/s; splitting them was measured slower). All fc
biases live in one resident [128, S*8] tile (per-style 16 B-row DMAs
fragment the DMA stream). gs tiles for the first STASH_S styles stay
resident in SBUF between phase 1 and their fc pass (no second HBM
read).
"""

import numpy as np

import concourse.bacc as bacc
import concourse.tile as tile
from concourse import mybir
from concourse.bass_utils import run_bass_kernel_spmd

S, D, BN, GH, AH, FCH = 18, 512, 32, 128, 16, 512
B = 8192
N_CORES = 8
BC = B // N_CORES          # samples per core
NB = 512                   # moving-dim (batch) tile = one fp32 PSUM bank
N_CHUNKS = BC // NB
GROUPS = [(0, 4), (4, 4), (8, 4), (12, 4), (16, 2)]
NG = len(GROUPS)
KT1 = 5                    # fc1 k-tiles: 4x gs(128) + gmh(128)
STASH_S = 14               # styles whose gs tiles stay resident across phases
PH1C1_AT = 4               # emit phase1(c1) after this style of the c0 pass

F32 = mybir.dt.float32
F16 = mybir.dt.float16
MM_DT = mybir.dt.float16
NP_MM = np.float16

_CACHE = {}


def build_program(rank2: bool = True):
    nc = bacc.Bacc("TRN2", target_bir_lowering=False, debug=False,
                   num_devices=N_CORES)
    mm = nc.tensor.matmul

    din = lambda name, shape, dt=MM_DT: nc.dram_tensor(name, shape, dt, kind="ExternalInput").ap()
    gsT = din("gsT", [S, D, BC])
    bn_w1t = din("bn_w1t", [128, S * 4 * BN])
    bn_b1g = din("bn_b1g", [128, NG], F32)
    vg = din("vg", [128, NG * GH])
    gm_b1p = din("gm_b1p", [GH, 1], F32)
    fc_w1t = din("fc_w1t", [S, 128, KT1 * FCH])     # [s, p, kt*512 + h]
    fc_w2t = din("fc_w2t", [S, 128, 16 * 128])      # [s, p, (kt*4+dt)*128 + j]
    fc_bt = din("fc_bt", [128, S * 8], F32)         # [p, s*8 + (b1:0-3 | b2:4-7)]
    if rank2:
        ageB = din("ageB", [128, BC])               # age broadcast over partitions
        d_all = din("d_all", [128, S * 4], F32)     # rank-1 age dirs, [p, s*4+ht]
    else:
        afT = din("afT", [AH, BC])
        fa_w = din("fa_w", [AH, S * 4 * 128])       # af k-tile weights [a, (s,ht,j)]
    yT = nc.dram_tensor("yT", [S, D, BC], F16, kind="ExternalOutput").ap()

    Relu = mybir.ActivationFunctionType.Relu
    ADD = mybir.AluOpType.add
    MULT = mybir.AluOpType.mult

    with (
        tile.TileContext(nc) as tc,
        tc.tile_pool(name="consts", bufs=1) as consts,
        tc.tile_pool(name="stash", bufs=1) as stash_pool,
        tc.tile_pool(name="gstr", bufs=2) as gstr_pool,       # streamed gs (styles >= STASH_S)
        tc.tile_pool(name="act1", bufs=3) as act1_pool,
        tc.tile_pool(name="wp", bufs=2) as w_pool,
        tc.tile_pool(name="y1p", bufs=2) as y1_pool,
        tc.tile_pool(name="outp", bufs=4) as out_pool,
        tc.tile_pool(name="ps", bufs=1, space="PSUM") as ps,
    ):
        # ---- per-style weight prefetch (gpsimd queue: idle at start) ----
        def load_w(s):
            w1s = w_pool.tile([128, KT1 * FCH], MM_DT, tag="w1", name=f"w1_{s}")
            nc.gpsimd.dma_start(w1s[:], fc_w1t[s, :, :])
            w2s = w_pool.tile([128, 16 * 128], MM_DT, tag="w2", name=f"w2_{s}")
            nc.gpsimd.dma_start(w2s[:], fc_w2t[s, :, :])
            return w1s, w2s

        w_cur = load_w(0)          # issued at t~0, lands during phase 1

        # ---- resident constants ----
        bn_w1_sb = consts.tile([128, S * 4 * BN], MM_DT, tag="bn_w1")
        nc.sync.dma_start(bn_w1_sb[:], bn_w1t[:])
        bn_b1_sb = consts.tile([128, NG], F32, tag="bn_b1")
        nc.sync.dma_start(bn_b1_sb[:], bn_b1g[:])
        vg_sb = consts.tile([128, NG * GH], MM_DT, tag="vg")
        nc.sync.dma_start(vg_sb[:], vg[:])
        gm_b1_sb = consts.tile([GH, 1], F32, tag="gm_b1")
        nc.sync.dma_start(gm_b1_sb[:], gm_b1p[:])
        fcb_sb = consts.tile([128, S * 8], F32, tag="fc_bt")
        nc.sync.dma_start(fcb_sb[:], fc_bt[:])
        if rank2:
            age_sb = consts.tile([128, BC], MM_DT, tag="ageB")
            nc.sync.dma_start(age_sb[:], ageB[:])
            d_sb = consts.tile([128, S * 4], F32, tag="d_all")
            nc.sync.dma_start(d_sb[:], d_all[:])
        else:
            af_sb = consts.tile([AH, BC], MM_DT, tag="af")
            nc.sync.dma_start(af_sb[:], afT[:])
            fa_sb = consts.tile([AH, S * 4 * 128], MM_DT, tag="fa_w")
            nc.sync.dma_start(fa_sb[:], fa_w[:])
        gmh_sb = [consts.tile([GH, NB], MM_DT, tag=f"gmh{c}", name=f"gmh{c}")
                  for c in range(N_CHUNKS)]

        def load_gs(s, c, pool, tag):
            b0 = c * NB
            t = pool.tile([128, 4 * NB], MM_DT, tag=tag, name=f"gs_{s}_{c}_{tag}")
            nc.sync.dma_start(
                t[:].rearrange("p (kt b) -> p kt b", kt=4),
                gsT[s, :, b0:b0 + NB].rearrange("(kt p) b -> p kt b", p=128))
            return t

        gs_tiles = {}      # (s, c) -> [128, 4*NB] tile

        def phase1(c):
            ps_g1 = ps.tile([GH, NB], F32, tag="g1", bufs=1, name=f"ps_g1_{c}")
            for gi, (s0, ng) in enumerate(GROUPS):
                pN = 32 * ng
                gts = []
                for j in range(ng):
                    s = s0 + j
                    pool, tag = ((stash_pool, f"gs_{s}_{c}") if s < STASH_S
                                 else (gstr_pool, "gsS"))
                    t = load_gs(s, c, pool, tag)
                    gs_tiles[(s, c)] = t
                    gts.append(t)
                ps_h1 = ps.tile([128, NB], F32, tag="h1", bufs=2,
                                name=f"ps_h1_{gi}_{c}")
                for kt in range(4):         # kt-major: col-groups concurrent
                    for j in range(ng):
                        s = s0 + j
                        mm(ps_h1[32 * j:32 * j + 32, :],
                           bn_w1_sb[:, (s * 4 + kt) * BN:(s * 4 + kt + 1) * BN],
                           gts[j][:, kt * NB:(kt + 1) * NB],
                           start=(kt == 0), stop=(kt == 3),
                           tile_position=(0, 32 * j))
                h1 = act1_pool.tile([128, NB], MM_DT, tag="h1s", name=f"h1_{gi}_{c}")
                nc.scalar.activation(h1[:pN, :], ps_h1[:pN, :], Relu,
                                     bias=bn_b1_sb[:pN, gi:gi + 1])
                mm(ps_g1[:], vg_sb[:pN, gi * GH:(gi + 1) * GH], h1[:pN, :],
                   start=(gi == 0), stop=(gi == NG - 1))
            nc.scalar.activation(gmh_sb[c][:], ps_g1[:], Relu, bias=gm_b1_sb[:])

        def fc_style(s, c, w1s, w2s):
            b0 = c * NB
            if s < STASH_S:
                gs_sb = gs_tiles[(s, c)]
            else:
                gs_sb = load_gs(s, c, gstr_pool, "gsS")
            y1 = []
            for ht in range(4):
                h0 = ht * 128
                ps_y1 = ps.tile([128, NB], F32, tag="y1", bufs=3,
                                name=f"ps_y1_{s}_{c}_{ht}")
                for kt in range(4):      # gs k-tiles first (no gmh dep)
                    mm(ps_y1[:],
                       w1s[:, kt * FCH + h0:kt * FCH + h0 + 128],
                       gs_sb[:, kt * NB:(kt + 1) * NB],
                       start=(kt == 0), stop=False)
                if not rank2:
                    mm(ps_y1[:],         # af k-tile (K=16)
                       fa_sb[:, (s * 4 + ht) * 128:(s * 4 + ht + 1) * 128],
                       af_sb[:, b0:b0 + NB],
                       start=False, stop=False)
                mm(ps_y1[:],             # gmh k-tile last
                   w1s[:, 4 * FCH + h0:4 * FCH + h0 + 128],
                   gmh_sb[c][:],
                   start=False, stop=True)
                if rank2:                # += age (x) d_s,ht  (rank-1, DVE)
                    nc.vector.scalar_tensor_tensor(
                        ps_y1[:], age_sb[:, b0:b0 + NB],
                        d_sb[:, s * 4 + ht:s * 4 + ht + 1],
                        ps_y1[:], op0=MULT, op1=ADD)
                y1t = y1_pool.tile([128, NB], MM_DT, tag=f"y1_{ht}",
                                   name=f"y1_{s}_{c}_{ht}")
                nc.scalar.activation(y1t[:], ps_y1[:], Relu,
                                     bias=fcb_sb[:, s * 8 + ht:s * 8 + ht + 1])
                y1.append(y1t)
            for dt_ in range(4):
                ps_y = ps.tile([128, NB], F32, tag="y", bufs=2,
                               name=f"ps_y_{s}_{c}_{dt_}")
                for kt in range(4):
                    mm(ps_y[:],
                       w2s[:, (kt * 4 + dt_) * 128:(kt * 4 + dt_ + 1) * 128],
                       y1[kt][:],
                       start=(kt == 0), stop=(kt == 3))
                o = out_pool.tile([128, NB], F16, tag="o", name=f"o_{s}_{c}_{dt_}")
                nc.vector.scalar_tensor_tensor(
                    o[:], ps_y[:], fcb_sb[:, s * 8 + 4 + dt_:s * 8 + 5 + dt_],
                    gs_sb[:, dt_ * NB:(dt_ + 1) * NB], op0=ADD, op1=ADD)
                nc.gpsimd.dma_start(yT[s, dt_ * 128:(dt_ + 1) * 128, b0:b0 + NB], o[:])

        # ---------------- schedule ----------------
        phase1(0)
        passes = [(0, s) for s in range(S)] + [(1, s) for s in range(S)]
        for idx, (c, s) in enumerate(passes):
            w_next = load_w(passes[idx + 1][1]) if idx + 1 < len(passes) else None
            fc_style(s, c, *w_cur)
            w_cur = w_next
            if c == 0 and s == PH1C1_AT:
                phase1(1)

    nc.compile()
    return nc


def _prep_weights(bn_w1, bn_b1, bn_w2, bn_b2, gm_w1, gm_b1, gm_w2, gm_b2,
                  fc_w1, fc_b1, fc_w2, fc_b2, c_age, c0_age, rank2):
    f = np.float32
    h = NP_MM
    # [p, (s, kt, j)] : bn_w1[s, kt*128+p, j]
    bn_w1t = np.ascontiguousarray(
        bn_w1.reshape(S, 4, 128, BN).transpose(2, 0, 1, 3).reshape(128, S * 4 * BN), h)
    bn_b1g = np.zeros((128, NG), f)
    for gi, (s0, ng) in enumerate(GROUPS):
        for j in range(ng):
            bn_b1g[32 * j:32 * j + 32, gi] = bn_b1[s0 + j]
    # fused bn2 @ gm_w1-block, group-stacked: vg[32j:+32, gi*128:+128] = V[s0+j]
    gm_w1b = gm_w1.reshape(S, BN, GH).astype(f)
    V = np.einsum('skm,smg->skg', bn_w2.astype(f), gm_w1b)      # [S, 32, 128]
    vg = np.zeros((128, NG * GH), h)
    for gi, (s0, ng) in enumerate(GROUPS):
        for j in range(ng):
            vg[32 * j:32 * j + 32, gi * GH:(gi + 1) * GH] = V[s0 + j]
    gm_b1p = (gm_b1.astype(f)
              + np.einsum('sm,smg->g', bn_b2.astype(f), gm_w1b)).reshape(GH, 1)

    W_g = fc_w1[:, :GH, :].astype(f)                            # [S, 128, 512]
    W_a = fc_w1[:, GH:GH + AH, :].astype(f)                     # [S, 16, 512]
    # fc1 rows: [gs (4x128) | gmh (gm_w2 @ W_g)]
    w1p = np.empty((S, KT1 * 128, FCH), f)
    w1p[:, :4 * 128] = fc_w1[:, GH + AH:]
    w1p[:, 4 * 128:] = np.einsum('kj,sjf->skf', gm_w2.astype(f), W_g)
    fc_w1t = np.ascontiguousarray(
        w1p.reshape(S, KT1, 128, FCH).transpose(0, 2, 1, 3).reshape(S, 128, KT1 * FCH), h)
    b1_full = (fc_b1.astype(f)
               + np.einsum('j,sjf->sf', gm_b2.astype(f), W_g))
    if rank2:
        b1_full = b1_full + np.einsum('a,saf->sf', c0_age, W_a)
    # biases combined: fc_bt[p, s*8 + t] = b1[s, t*128+p] (t<4) | b2[s, (t-4)*128+p]
    fc_bt = np.empty((128, S * 8), f)
    fc_bt.reshape(128, S, 8)[:, :, :4] = b1_full.reshape(S, 4, 128).transpose(2, 0, 1)
    fc_bt.reshape(128, S, 8)[:, :, 4:] = fc_b2.astype(f).reshape(S, 4, 128).transpose(2, 0, 1)
    fc_w2t = np.ascontiguousarray(
        fc_w2.reshape(S, 4, 128, 4, 128).transpose(0, 2, 1, 3, 4).reshape(S, 128, 16 * 128), h)
    out = dict(bn_w1t=bn_w1t, bn_b1g=bn_b1g, vg=vg, gm_b1p=gm_b1p,
               fc_w1t=fc_w1t, fc_w2t=fc_w2t, fc_bt=fc_bt)
    if rank2:
        d = np.einsum('a,saf->sf', c_age, W_a)                  # [S, 512]
        out["d_all"] = np.ascontiguousarray(
            d.reshape(S, 4, 128).transpose(2, 0, 1).reshape(128, S * 4), f)
    else:
        # af k-tile weights: fa_w[a, (s*4+ht)*128 + j] = W_a[s, a, ht*128+j]
        out["fa_w"] = np.ascontiguousarray(
            W_a.reshape(S, AH, 4, 128).transpose(1, 0, 2, 3).reshape(AH, S * 4 * 128), h)
    return out


def run(inputs: dict, trace: bool = False):
    """Build in_maps from full inputs, run SPMD on 8 cores, return
    (full_output, BassKernelResults)."""
    gs = inputs["global_styles"]
    ages = np.asarray(inputs["target_ages"], np.float32)
    # host: exact fp32 age MLP (tiny), then affine-in-age fit
    af = np.maximum(ages[:, None] @ inputs["age_w1"] + inputs["age_b1"], 0.0)
    af = (af @ inputs["age_w2"] + inputs["age_b2"]).astype(np.float32)  # [B, 16]
    A = np.stack([ages, np.ones_like(ages)], axis=1)                    # [B, 2]
    sol, *_ = np.linalg.lstsq(A.astype(np.float64), af.astype(np.float64),
                              rcond=None)
    resid = np.abs(af - (A @ sol.astype(np.float32))).max()
    rank2 = bool(resid <= 1e-4 * max(1.0, np.abs(af).max()))
    c_age, c0_age = sol[0].astype(np.float32), sol[1].astype(np.float32)

    key = ("nc", rank2)
    if key not in _CACHE:
        _CACHE[key] = build_program(rank2)
    nc = _CACHE[key]

    w = _prep_weights(
        inputs["bn_w1"], inputs["bn_b1"], inputs["bn_w2"], inputs["bn_b2"],
        inputs["gm_w1"], inputs["gm_b1"], inputs["gm_w2"], inputs["gm_b2"],
        inputs["fc_w1"], inputs["fc_b1"], inputs["fc_w2"], inputs["fc_b2"],
        c_age, c0_age, rank2)

    gsT_full = np.ascontiguousarray(gs.transpose(1, 2, 0).astype(NP_MM))  # [S, D, B]
    ages16 = ages.astype(NP_MM)
    afT_full = np.ascontiguousarray(af.T.astype(NP_MM))                   # [16, B]
    in_maps = []
    for c in range(N_CORES):
        sl = slice(c * BC, (c + 1) * BC)
        m = dict(w)
        m["gsT"] = np.ascontiguousarray(gsT_full[:, :, sl])
        if rank2:
            m["ageB"] = np.ascontiguousarray(
                np.broadcast_to(ages16[None, sl], (128, BC)))
        else:
            m["afT"] = np.ascontiguousarray(afT_full[:, sl])
        in_maps.append(m)

    res = run_bass_kernel_spmd(nc, in_maps, core_ids=list(range(N_CORES)),
                               trace=trace)
    yT = np.concatenate([res.results[c]["yT"][:, :, :] for c in range(N_CORES)],
                        axis=2)                              # [S, D, B] fp16
    y = yT.astype(np.float32).transpose(2, 0, 1)             # [B, S, D]
    return np.ascontiguousarray(y), res


def kernel(**inputs) -> np.ndarray:
    y, _ = run(inputs, trace=False)
    return y


# revision 5
# speedup vs baseline: 1.1924x; 1.0409x over previous
"""Trainium2 Bass kernel for nn_Blender (per-style MLP blender).

Strategy
--------
Pure data parallel over the batch: each of the 8 NeuronCores processes
B/8 = 1024 samples with a full replica of the weights. No collectives.

On-chip layout is feature-major ([features -> partitions, batch -> free
dim]) so every GEMM contracts along the partition axis with batch as the
moving dim (N=512 = one fp32 PSUM bank). The host pre-transposes
global_styles to [S, D, B] (fp16) and post-transposes the output back,
so all device DMA is contiguous. Output is written fp16 (adds ~5e-4
max-rel-err; tolerance is 2e-2).

GEMMs run in fp16 (1 cycle/row) accumulating into fp32 PSUM; epilogues
(bias/relu/residual) run on ACT/DVE. Every fc k-tile is a full 128-row
weight load so the PE preloads it into the background weight buffer
while the previous matmul streams (partial-row LDWEIGHTS cannot be
pulled ahead and stall the PE ~400ns each -- measured).

Algebraic folds (all exact):
  * bn2 + gm1 fuse: gm1_in = concat_s(h1_s @ bn_w2_s + bn_b2_s), so
    gm1_out = sum_s h1_s @ (bn_w2_s @ gm_w1_block_s) + folded bias.
    One block-stacked K=128 GEMM per 4-style group replaces bn2+gm1.
  * gm2 fold: fc1 consumes gmh (the gm hidden) directly through
    W_g' = gm_w2 @ fc_w1[:, :GH]; gm_b2's term goes into fc1's bias.
  * age rank-2: with age_b1 == 0 and ages >= 0 the age MLP is exactly
    affine in the scalar age: af = age*c + c0 (validated at runtime via
    lstsq; falls back to an explicit K=16 k-tile otherwise). Its fc1
    contribution is a rank-1 outer product age (x) d_s added into PSUM
    by the (otherwise idle) Vector engine, and a bias fold.

Schedule (chunk-outer, BC=1024 samples in 2 chunks of NB=512):
  phase1(c0)            -- DMA-paced front: 9.4 MB of gs for chunk 0
  fc pass c0, s=0..17   -- PE-bound; per-style weights double-buffered
    (phase1(c1) emitted after s=4: its gs DMAs stream during the c0
     pass; ~6us of PE work dropped into a PE-bound region)
  fc pass c1, s=0..17
Weight DMAs ride the gpsimd queue (idle early -> style-0 weights land
during phase 1; sync queue is saturated issuing gs loads at kernel
start). gs loads are one 3D-AP DMA per [128, 4*NB] tile (4 KB/partition
packets sustain ~385 GB/s; splitting them was measured slower). All fc
biases live in one resident [128, S*8] tile (per-style 16 B-row DMAs
fragment the DMA stream). gs tiles for the first STASH_S styles stay
resident in SBUF between phase 1 and their fc pass (no second HBM
read).
"""

import numpy as np

import concourse.bacc as bacc
import concourse.tile as tile
from concourse import mybir
from concourse.bass_utils import run_bass_kernel_spmd

S, D, BN, GH, AH, FCH = 18, 512, 32, 128, 16, 512
B = 8192
N_CORES = 8
BC = B // N_CORES          # samples per core
NB = 512                   # moving-dim (batch) tile = one fp32 PSUM bank
N_CHUNKS = BC // NB
GROUPS = [(0, 4), (4, 4), (8, 4), (12, 4), (16, 2)]
NG = len(GROUPS)
KT1 = 5                    # fc1 k-tiles: 4x gs(128) + gmh(128)
STASH_S = 14               # styles whose gs tiles stay resident across phases
PH1C1_AT = 5               # emit phase1(c1) after this style of the c0 pass

F32 = mybir.dt.float32
F16 = mybir.dt.float16
MM_DT = mybir.dt.float16
NP_MM = np.float16

_CACHE = {}


def build_program(rank2: bool = True):
    nc = bacc.Bacc("TRN2", target_bir_lowering=False, debug=False,
                   num_devices=N_CORES)
    mm = nc.tensor.matmul

    din = lambda name, shape, dt=MM_DT: nc.dram_tensor(name, shape, dt, kind="ExternalInput").ap()
    gsT = din("gsT", [S, D, BC])
    bn_w1t = din("bn_w1t", [128, S * 4 * BN])
    bn_b1g = din("bn_b1g", [128, NG], F32)
    vg = din("vg", [128, NG * GH])
    gm_b1p = din("gm_b1p", [GH, 1], F32)
    fc_w1t = din("fc_w1t", [S, 128, KT1 * FCH])     # [s, p, kt*512 + h]
    fc_w2t = din("fc_w2t", [S, 128, 16 * 128])      # [s, p, (kt*4+dt)*128 + j]
    fc_bt = din("fc_bt", [128, S * 8], F32)         # [p, s*8 + (b1:0-3 | b2:4-7)]
    if rank2:
        ageB = din("ageB", [128, BC])               # age broadcast over partitions
        d_all = din("d_all", [128, S * 4], F32)     # rank-1 age dirs, [p, s*4+ht]
    else:
        afT = din("afT", [AH, BC])
        fa_w = din("fa_w", [AH, S * 4 * 128])       # af k-tile weights [a, (s,ht,j)]
    yT = nc.dram_tensor("yT", [S, D, BC], F16, kind="ExternalOutput").ap()

    Relu = mybir.ActivationFunctionType.Relu
    ADD = mybir.AluOpType.add
    MULT = mybir.AluOpType.mult

    with (
        tile.TileContext(nc) as tc,
        tc.tile_pool(name="consts", bufs=1) as consts,
        tc.tile_pool(name="stash", bufs=1) as stash_pool,
        tc.tile_pool(name="gstr", bufs=2) as gstr_pool,       # streamed gs (styles >= STASH_S)
        tc.tile_pool(name="act1", bufs=3) as act1_pool,
        tc.tile_pool(name="wp", bufs=2) as w_pool,
        tc.tile_pool(name="y1p", bufs=2) as y1_pool,
        tc.tile_pool(name="outp", bufs=4) as out_pool,
        tc.tile_pool(name="ps", bufs=1, space="PSUM") as ps,
    ):
        # ---- per-style weight prefetch ----
        # w(0) rides the gpsimd ring (empty at t=0 -> lands during phase 1);
        # later styles ride the sync ring (idle once phase-1 gs issues drain).
        # The out-stores get the gpsimd ring to themselves afterwards, so the
        # final DMA drain only waits on the last few output tiles.
        _wn = [0]
        def load_w(s, eng=None):
            _wn[0] += 1
            w1s = w_pool.tile([128, KT1 * FCH], MM_DT, tag="w1", bufs=3,
                              name=f"w1_{_wn[0]}_{s}")
            (eng or nc.sync).dma_start(w1s[:], fc_w1t[s, :, :])
            w2s = w_pool.tile([128, 16 * 128], MM_DT, tag="w2", bufs=3,
                              name=f"w2_{_wn[0]}_{s}")
            (eng or nc.sync).dma_start(w2s[:], fc_w2t[s, :, :])
            return w1s, w2s

        w0 = load_w(0, eng=nc.gpsimd)   # issued at t~0, lands during phase 1

        # ---- resident constants ----
        bn_w1_sb = consts.tile([128, S * 4 * BN], MM_DT, tag="bn_w1")
        nc.sync.dma_start(bn_w1_sb[:], bn_w1t[:])
        bn_b1_sb = consts.tile([128, NG], F32, tag="bn_b1")
        nc.sync.dma_start(bn_b1_sb[:], bn_b1g[:])
        vg_sb = consts.tile([128, NG * GH], MM_DT, tag="vg")
        nc.sync.dma_start(vg_sb[:], vg[:])
        gm_b1_sb = consts.tile([GH, 1], F32, tag="gm_b1")
        nc.sync.dma_start(gm_b1_sb[:], gm_b1p[:])
        fcb_sb = consts.tile([128, S * 8], F32, tag="fc_bt")
        nc.sync.dma_start(fcb_sb[:], fc_bt[:])
        if rank2:
            age_sb = consts.tile([128, BC], MM_DT, tag="ageB")
            nc.sync.dma_start(age_sb[:], ageB[:])
            d_sb = consts.tile([128, S * 4], F32, tag="d_all")
            nc.sync.dma_start(d_sb[:], d_all[:])
        else:
            af_sb = consts.tile([AH, BC], MM_DT, tag="af")
            nc.sync.dma_start(af_sb[:], afT[:])
            fa_sb = consts.tile([AH, S * 4 * 128], MM_DT, tag="fa_w")
            nc.sync.dma_start(fa_sb[:], fa_w[:])
        gmh_sb = [consts.tile([GH, NB], MM_DT, tag=f"gmh{c}", name=f"gmh{c}")
                  for c in range(N_CHUNKS)]

        def load_gs(s, c, pool, tag):
            b0 = c * NB
            t = pool.tile([128, 4 * NB], MM_DT, tag=tag, name=f"gs_{s}_{c}_{tag}")
            nc.sync.dma_start(
                t[:].rearrange("p (kt b) -> p kt b", kt=4),
                gsT[s, :, b0:b0 + NB].rearrange("(kt p) b -> p kt b", p=128))
            return t

        gs_tiles = {}      # (s, c) -> [128, 4*NB] tile

        def phase1(c):
            ps_g1 = ps.tile([GH, NB], F32, tag="g1", bufs=1, name=f"ps_g1_{c}")
            for gi, (s0, ng) in enumerate(GROUPS):
                pN = 32 * ng
                gts = []
                for j in range(ng):
                    s = s0 + j
                    pool, tag = ((stash_pool, f"gs_{s}_{c}") if s < STASH_S
                                 else (gstr_pool, "gsS"))
                    t = load_gs(s, c, pool, tag)
                    gs_tiles[(s, c)] = t
                    gts.append(t)
                ps_h1 = ps.tile([128, NB], F32, tag="y1", bufs=3,
                                name=f"ps_h1_{gi}_{c}")
                for kt in range(4):         # kt-major: col-groups concurrent
                    for j in range(ng):
                        s = s0 + j
                        mm(ps_h1[32 * j:32 * j + 32, :],
                           bn_w1_sb[:, (s * 4 + kt) * BN:(s * 4 + kt + 1) * BN],
                           gts[j][:, kt * NB:(kt + 1) * NB],
                           start=(kt == 0), stop=(kt == 3),
                           tile_position=(0, 32 * j))
                h1 = act1_pool.tile([128, NB], MM_DT, tag="h1s", name=f"h1_{gi}_{c}")
                nc.scalar.activation(h1[:pN, :], ps_h1[:pN, :], Relu,
                                     bias=bn_b1_sb[:pN, gi:gi + 1])
                mm(ps_g1[:], vg_sb[:pN, gi * GH:(gi + 1) * GH], h1[:pN, :],
                   start=(gi == 0), stop=(gi == NG - 1))
            nc.scalar.activation(gmh_sb[c][:], ps_g1[:], Relu, bias=gm_b1_sb[:])

        def fc_style(s, c, w1s, w2s):
            b0 = c * NB
            if s < STASH_S:
                gs_sb = gs_tiles[(s, c)]
            else:
                gs_sb = load_gs(s, c, gstr_pool, "gsS")
            y1 = []
            for ht in range(4):
                h0 = ht * 128
                ps_y1 = ps.tile([128, NB], F32, tag="y1", bufs=3,
                                name=f"ps_y1_{s}_{c}_{ht}")
                for kt in range(4):      # gs k-tiles first (no gmh dep)
                    mm(ps_y1[:],
                       w1s[:, kt * FCH + h0:kt * FCH + h0 + 128],
                       gs_sb[:, kt * NB:(kt + 1) * NB],
                       start=(kt == 0), stop=False)
                if not rank2:
                    mm(ps_y1[:],         # af k-tile (K=16)
                       fa_sb[:, (s * 4 + ht) * 128:(s * 4 + ht + 1) * 128],
                       af_sb[:, b0:b0 + NB],
                       start=False, stop=False)
                mm(ps_y1[:],             # gmh k-tile last
                   w1s[:, 4 * FCH + h0:4 * FCH + h0 + 128],
                   gmh_sb[c][:],
                   start=False, stop=True)
                if rank2:                # += age (x) d_s,ht  (rank-1, DVE)
                    nc.vector.scalar_tensor_tensor(
                        ps_y1[:], age_sb[:, b0:b0 + NB],
                        d_sb[:, s * 4 + ht:s * 4 + ht + 1],
                        ps_y1[:], op0=MULT, op1=ADD)
                y1t = y1_pool.tile([128, NB], MM_DT, tag=f"y1_{ht}",
                                   name=f"y1_{s}_{c}_{ht}")
                nc.scalar.activation(y1t[:], ps_y1[:], Relu,
                                     bias=fcb_sb[:, s * 8 + ht:s * 8 + ht + 1])
                y1.append(y1t)
            for dt_ in range(4):
                ps_y = ps.tile([128, NB], F32, tag="y", bufs=4,
                               name=f"ps_y_{s}_{c}_{dt_}")
                for kt in range(4):
                    mm(ps_y[:],
                       w2s[:, (kt * 4 + dt_) * 128:(kt * 4 + dt_ + 1) * 128],
                       y1[kt][:],
                       start=(kt == 0), stop=(kt == 3))
                o = out_pool.tile([128, NB], F16, tag="o", name=f"o_{s}_{c}_{dt_}")
                nc.vector.scalar_tensor_tensor(
                    o[:], ps_y[:], fcb_sb[:, s * 8 + 4 + dt_:s * 8 + 5 + dt_],
                    gs_sb[:, dt_ * NB:(dt_ + 1) * NB], op0=ADD, op1=ADD)
                nc.gpsimd.dma_start(yT[s, dt_ * 128:(dt_ + 1) * 128, b0:b0 + NB], o[:])

        # ---------------- schedule ----------------
        phase1(0)
        passes = [(0, s) for s in range(S)] + [(1, s) for s in range(S)]
        w_q = [w0, load_w(passes[1][1])]
        for idx, (c, s) in enumerate(passes):
            if idx + 2 < len(passes):
                w_q.append(load_w(passes[idx + 2][1]))
            fc_style(s, c, *w_q.pop(0))
            if c == 0 and s == PH1C1_AT:
                phase1(1)

    nc.compile()
    return nc


def _prep_weights(bn_w1, bn_b1, bn_w2, bn_b2, gm_w1, gm_b1, gm_w2, gm_b2,
                  fc_w1, fc_b1, fc_w2, fc_b2, c_age, c0_age, rank2):
    f = np.float32
    h = NP_MM
    # [p, (s, kt, j)] : bn_w1[s, kt*128+p, j]
    bn_w1t = np.ascontiguousarray(
        bn_w1.reshape(S, 4, 128, BN).transpose(2, 0, 1, 3).reshape(128, S * 4 * BN), h)
    bn_b1g = np.zeros((128, NG), f)
    for gi, (s0, ng) in enumerate(GROUPS):
        for j in range(ng):
            bn_b1g[32 * j:32 * j + 32, gi] = bn_b1[s0 + j]
    # fused bn2 @ gm_w1-block, group-stacked: vg[32j:+32, gi*128:+128] = V[s0+j]
    gm_w1b = gm_w1.reshape(S, BN, GH).astype(f)
    V = np.einsum('skm,smg->skg', bn_w2.astype(f), gm_w1b)      # [S, 32, 128]
    vg = np.zeros((128, NG * GH), h)
    for gi, (s0, ng) in enumerate(GROUPS):
        for j in range(ng):
            vg[32 * j:32 * j + 32, gi * GH:(gi + 1) * GH] = V[s0 + j]
    gm_b1p = (gm_b1.astype(f)
              + np.einsum('sm,smg->g', bn_b2.astype(f), gm_w1b)).reshape(GH, 1)

    W_g = fc_w1[:, :GH, :].astype(f)                            # [S, 128, 512]
    W_a = fc_w1[:, GH:GH + AH, :].astype(f)                     # [S, 16, 512]
    # fc1 rows: [gs (4x128) | gmh (gm_w2 @ W_g)]
    w1p = np.empty((S, KT1 * 128, FCH), f)
    w1p[:, :4 * 128] = fc_w1[:, GH + AH:]
    w1p[:, 4 * 128:] = np.einsum('kj,sjf->skf', gm_w2.astype(f), W_g)
    fc_w1t = np.ascontiguousarray(
        w1p.reshape(S, KT1, 128, FCH).transpose(0, 2, 1, 3).reshape(S, 128, KT1 * FCH), h)
    b1_full = (fc_b1.astype(f)
               + np.einsum('j,sjf->sf', gm_b2.astype(f), W_g))
    if rank2:
        b1_full = b1_full + np.einsum('a,saf->sf', c0_age, W_a)
    # biases combined: fc_bt[p, s*8 + t] = b1[s, t*128+p] (t<4) | b2[s, (t-4)*128+p]
    fc_bt = np.empty((128, S * 8), f)
    fc_bt.reshape(128, S, 8)[:, :, :4] = b1_full.reshape(S, 4, 128).transpose(2, 0, 1)
    fc_bt.reshape(128, S, 8)[:, :, 4:] = fc_b2.astype(f).reshape(S, 4, 128).transpose(2, 0, 1)
    fc_w2t = np.ascontiguousarray(
        fc_w2.reshape(S, 4, 128, 4, 128).transpose(0, 2, 1, 3, 4).reshape(S, 128, 16 * 128), h)
    out = dict(bn_w1t=bn_w1t, bn_b1g=bn_b1g, vg=vg, gm_b1p=gm_b1p,
               fc_w1t=fc_w1t, fc_w2t=fc_w2t, fc_bt=fc_bt)
    if rank2:
        d = np.einsum('a,saf->sf', c_age, W_a)                  # [S, 512]
        out["d_all"] = np.ascontiguousarray(
            d.reshape(S, 4, 128).transpose(2, 0, 1).reshape(128, S * 4), f)
    else:
        # af k-tile weights: fa_w[a, (s*4+ht)*128 + j] = W_a[s, a, ht*128+j]
        out["fa_w"] = np.ascontiguousarray(
            W_a.reshape(S, AH, 4, 128).transpose(1, 0, 2, 3).reshape(AH, S * 4 * 128), h)
    return out


def run(inputs: dict, trace: bool = False):
    """Build in_maps from full inputs, run SPMD on 8 cores, return
    (full_output, BassKernelResults)."""
    gs = inputs["global_styles"]
    ages = np.asarray(inputs["target_ages"], np.float32)
    # host: exact fp32 age MLP (tiny), then affine-in-age fit
    af = np.maximum(ages[:, None] @ inputs["age_w1"] + inputs["age_b1"], 0.0)
    af = (af @ inputs["age_w2"] + inputs["age_b2"]).astype(np.float32)  # [B, 16]
    A = np.stack([ages, np.ones_like(ages)], axis=1)                    # [B, 2]
    sol, *_ = np.linalg.lstsq(A.astype(np.float64), af.astype(np.float64),
                              rcond=None)
    resid = np.abs(af - (A @ sol.astype(np.float32))).max()
    rank2 = bool(resid <= 1e-4 * max(1.0, np.abs(af).max()))
    c_age, c0_age = sol[0].astype(np.float32), sol[1].astype(np.float32)

    key = ("nc", rank2)
    if key not in _CACHE:
        _CACHE[key] = build_program(rank2)
    nc = _CACHE[key]

    w = _prep_weights(
        inputs["bn_w1"], inputs["bn_b1"], inputs["bn_w2"], inputs["bn_b2"],
        inputs["gm_w1"], inputs["gm_b1"], inputs["gm_w2"], inputs["gm_b2"],
        inputs["fc_w1"], inputs["fc_b1"], inputs["fc_w2"], inputs["fc_b2"],
        c_age, c0_age, rank2)

    gsT_full = np.ascontiguousarray(gs.transpose(1, 2, 0).astype(NP_MM))  # [S, D, B]
    ages16 = ages.astype(NP_MM)
    afT_full = np.ascontiguousarray(af.T.astype(NP_MM))                   # [16, B]
    in_maps = []
    for c in range(N_CORES):
        sl = slice(c * BC, (c + 1) * BC)
        m = dict(w)
        m["gsT"] = np.ascontiguousarray(gsT_full[:, :, sl])
        if rank2:
            m["ageB"] = np.ascontiguousarray(
                np.broadcast_to(ages16[None, sl], (128, BC)))
        else:
            m["afT"] = np.ascontiguousarray(afT_full[:, sl])
        in_maps.append(m)

    res = run_bass_kernel_spmd(nc, in_maps, core_ids=list(range(N_CORES)),
                               trace=trace)
    yT = np.concatenate([res.results[c]["yT"][:, :, :] for c in range(N_CORES)],
                        axis=2)                              # [S, D, B] fp16
    y = yT.astype(np.float32).transpose(2, 0, 1)             # [B, S, D]
    return np.ascontiguousarray(y), res


def kernel(**inputs) -> np.ndarray:
    y, _ = run(inputs, trace=False)
    return y
